# revision 1
# baseline (speedup 1.0000x reference)
"""Bayesian-embedding lookup (BBBEmbedding) Trainium2 kernel, 8 NeuronCores.

reference:
    sampled = W_mu + log1p(exp(W_rho)) * clip(eps, -10, 10)   # [V, D]
    out     = sampled[x]                                      # [B, L, D]

Strategy (model-parallel row sharding + run-length block gather):
  - Row-shard the three [V, D] tables across the 8 cores (VS = V/8 rows,
    padded to VSP = 12544 = 98*128 so the flat [128, VSP] view holds exactly
    98 whole rows per SBUF partition).
  - Each core computes its sampled-table shard once (ScalarE exp/ln +
    VectorE mul/add), replicates each row 4x (VectorE copies) and writes a
    [*, 4*D] "x4" table to DRAM scratch with 2KB-contiguous descriptors.
    The x4 table is split into two half tensors (rows with q = r mod 98
    below/above 49) so gathers against the first half can start while the
    second half is still being computed (phase overlap).
  - Host sorts the B*L token indices (stable argsort = bucket by owning
    core AND by row within the bucket) and run-length encodes each core's
    bucket into three gather streams against the x4 table: blocks of 4
    tokens (2048B elements), pairs (1024B: the first half of an x4 entry),
    and singles (512B). This exploits the ~8x average row multiplicity to
    cut DMA-gather descriptor generation (the Q7/SWDGE per-index cost,
    ~8 ns/index, is the kernel's bottleneck) ~2.7x vs per-token gathering,
    with zero gathered-byte inflation.
  - Each core DMA-gathers its blocks (InstDMAGatherAnt) and streams them to
    compact per-stream outputs; the host scatters the slots back to token
    order.
"""

import math

import numpy as np

V = 100000
D = 128  # row = 512 bytes; layout tricks below assume D == 128
NCORES = 8
VS = V // NCORES  # 12500 table rows per core
VSP = 12544  # padded shard rows = 98 * 128
RPP = VSP // 128  # rows per partition in the flat view (98)
HQ = RPP // 2  # rows per partition per half (49)
RB = 4  # replication factor of the x4 table == tokens per full block
STREAMS = (4, 2, 1)  # block sizes; must be exactly this for the RLE below
TBS = {4: 1024, 2: 512, 1: 512}  # gather blocks per tile per stream
ROWS_PER_AT = 7  # phase-A tile rows per partition (divides HQ = 49)

_nc_cache: dict = {}

# Debug/profiling knobs (unused by the grading path: TRACE defaults False).
TRACE = False
LAST_PROFILE: dict = {}


def _build_nc(nbps, vsp=VSP, tbs=TBS, rows_at=ROWS_PER_AT, num_devices=NCORES):
    """Build + compile the per-core Bass program.

    nbps: {block_size: (nbp_lo, nbp_hi)} padded per-half block counts.
    """
    import concourse.bacc as bacc
    import concourse.bass as bass
    import concourse.tile as tile
    from concourse import mybir

    f32 = mybir.dt.float32
    i16 = mybir.dt.int16
    rpp = vsp // 128
    hq = rpp // 2
    vsph = vsp // 2  # rows per half
    fa = rows_at * D
    nat = hq // rows_at  # phase-A tiles per half
    assert rpp % 2 == 0 and hq % rows_at == 0
    assert all(
        lo % tbs[bs] == 0 and hi % tbs[bs] == 0 for bs, (lo, hi) in nbps.items()
    )

    nc = bacc.Bacc(
        "TRN2", target_bir_lowering=False, debug=False, num_devices=num_devices
    )
    # Flat [128, vsp] view of the [vsp, D] tables: partition p holds rows
    # [p*rpp, (p+1)*rpp) — whole rows, since vsp = 128*rpp and D == 128.
    mu_d = nc.dram_tensor("mu", [128, vsp], f32, kind="ExternalInput").ap()
    rho_d = nc.dram_tensor("rho", [128, vsp], f32, kind="ExternalInput").ap()
    eps_d = nc.dram_tensor("eps", [128, vsp], f32, kind="ExternalInput").ap()
    # Per-stream block row-ids (lo half then hi half); block j lives at
    # idx[16k + j % 16, j // 16] for each replicated 16-partition stripe.
    idx_d = {
        bs: nc.dram_tensor(
            f"idx{bs}", [128, (lo + hi) // 16], i16, kind="ExternalInput"
        ).ap()
        for bs, (lo, hi) in nbps.items()
    }
    out_d = {
        bs: nc.dram_tensor(
            f"out{bs}", [lo + hi, bs * D], f32, kind="ExternalOutput"
        ).ap()
        for bs, (lo, hi) in nbps.items()
    }
    # Half x4 tables: local row rh (= p*hq + q') replicated RB times at byte
    # offset rh*RB*D*4. Viewed [128, hq, RB, D] for phase-A writes.
    samp_h = [nc.dram_tensor(f"samp4_{h}", [128, hq, RB, D], f32).ap() for h in (0, 1)]
    gather_src = {
        (h, bs): bass.AP(
            tensor=samp_h[h].tensor, offset=0, ap=[[RB * D, vsph], [1, bs * D]]
        )
        for h in (0, 1)
        for bs in nbps
    }

    with tile.TileContext(nc) as tc:
        with (
            tc.tile_pool(name="phase_a", bufs=3) as ap_pool,
            tc.tile_pool(name="phase_b4", bufs=4) as b4_pool,
            tc.tile_pool(name="phase_b", bufs=3) as b_pool,
            tc.tile_pool(name="phase_b_idx", bufs=1) as bi_pool,
        ):
            # Preload every stream's block-row-ids up front (tiny, and first
            # in the sync HWDGE FIFO so gathers never wait on idx data).
            idx_t = {}
            for bs, (lo, hi) in nbps.items():
                t = bi_pool.tile([128, (lo + hi) // 16], i16, tag=f"idx{bs}")
                nc.sync.dma_start(out=t[:], in_=idx_d[bs][:])
                idx_t[bs] = t
            # Phase A: sampled = mu + ln(exp(rho) + 1) * clip(eps, +-10), x4,
            # lo half (tiles 0..nat-1) then hi half.
            for j in range(2 * nat):
                h, jh = divmod(j, nat)
                sl = slice((h * hq + jh * rows_at) * D, (h * hq + (jh + 1) * rows_at) * D)
                qsl = slice(jh * rows_at, (jh + 1) * rows_at)
                mu_t = ap_pool.tile([128, fa], f32)
                rho_t = ap_pool.tile([128, fa], f32)
                eps_t = ap_pool.tile([128, fa], f32)
                sig_t = ap_pool.tile([128, fa], f32)
                rep_t = ap_pool.tile([128, rows_at, RB, D], f32)
                # Spread phase-A load issue across engines: sync HWDGE would
                # serialize all issues in one FIFO; Pool (SWDGE) is idle here.
                nc.sync.dma_start(out=mu_t[:], in_=mu_d[:, sl])
                nc.gpsimd.dma_start(out=rho_t[:], in_=rho_d[:, sl])
                nc.gpsimd.dma_start(out=eps_t[:], in_=eps_d[:, sl])
                nc.scalar.activation(
                    out=sig_t[:], in_=rho_t[:], func=mybir.ActivationFunctionType.Exp
                )
                nc.scalar.activation(
                    out=sig_t[:],
                    in_=sig_t[:],
                    func=mybir.ActivationFunctionType.Ln,
                    bias=1.0,
                )
                nc.vector.tensor_scalar(
                    out=eps_t[:],
                    in0=eps_t[:],
                    scalar1=10.0,
                    scalar2=-10.0,
                    op0=mybir.AluOpType.min,
                    op1=mybir.AluOpType.max,
                )
                nc.vector.tensor_tensor(
                    out=sig_t[:], in0=sig_t[:], in1=eps_t[:], op=mybir.AluOpType.mult
                )
                nc.vector.tensor_tensor(
                    out=sig_t[:], in0=sig_t[:], in1=mu_t[:], op=mybir.AluOpType.add
                )
                sig_ap = sig_t[:]
                sig_bcast = bass.AP(
                    tensor=sig_ap.tensor,
                    offset=sig_ap.offset,
                    ap=[sig_ap.ap[0], [D, rows_at], [0, RB], [1, D]],
                )
                nc.vector.tensor_copy(rep_t[:], sig_bcast)
                nc.sync.dma_start(out=samp_h[h][:, qsl, :, :], in_=rep_t[:])

            # Phase B: per-stream block gathers from the x4 halves, lo tiles
            # first (they only depend on the lo half of phase A), streams
            # round-robin so Pool desc-gen interleaves with big transfers.
            def tiles_of(h):
                seq = []
                for bs, (lo, hi) in nbps.items():
                    n0 = 0 if h == 0 else lo // tbs[bs]
                    cnt = (lo if h == 0 else hi) // tbs[bs]
                    seq.append([(bs, n0 + g) for g in range(cnt)])
                order = []
                while any(seq):
                    for s in seq:
                        if s:
                            order.append(s.pop(0))
                return order

            for h in (0, 1):
                for bs, g in tiles_of(h):
                    tb = tbs[bs]
                    csl = slice(g * (tb // 16), (g + 1) * (tb // 16))
                    g_t = (b4_pool if bs == 4 else b_pool).tile(
                        [128, tb // 128, bs * D], f32, tag=f"g{bs}"
                    )
                    nc.gpsimd.dma_gather(
                        g_t[:],
                        gather_src[(h, bs)],
                        idx_t[bs][:, csl],
                        tb,
                        tb,
                        bs * D,
                        elem_step=RB * D,
                        single_packet=False,
                    )
                    # Scalar-engine HWDGE ring: out-writes must not queue
                    # behind phase A's transfers in the sync FIFO.
                    nc.scalar.dma_start(
                        out=out_d[bs][g * tb : (g + 1) * tb].rearrange(
                            "(c p) e -> p c e", p=128
                        ),
                        in_=g_t[:],
                    )

    nc.compile()
    return nc


def _get_nc(nbps):
    key = tuple(sorted(nbps.items()))
    nc = _nc_cache.get(key)
    if nc is None:
        nc = _build_nc(nbps)
        _nc_cache[key] = nc
    return nc


def _encode_blocks(seg):
    """RLE a sorted local-row array into gather blocks of sizes (4, 2, 1).

    Returns {bs: (blk_u, tok_sel, tok_block, tok_within)}: blk_u[b] = shard
    row id of stream-bs block b; tokens seg[tok_sel] sit at offset
    tok_within of block tok_block (ordinal within the stream).
    """
    u, k = np.unique(seg, return_counts=True)
    n = seg.size
    run_start = np.zeros(k.size + 1, dtype=np.int64)
    np.cumsum(k, out=run_start[1:])
    r_tok = np.repeat(np.arange(k.size, dtype=np.int64), k)
    o = np.arange(n, dtype=np.int64) - run_start[:-1][r_tok]
    out = {}
    a = k >> 2  # full 4-blocks per run
    b = (k & 3) >> 1  # 2-blocks per run (0 or 1)
    c = k & 1  # singles per run (0 or 1)
    for bs, nb in ((4, a), (2, b), (1, c)):
        base = np.zeros(nb.size + 1, dtype=np.int64)
        np.cumsum(nb, out=base[1:])
        if bs == 4:
            sel = o < 4 * a[r_tok]
            off = o[sel]
        elif bs == 2:
            sel = (o >= 4 * a[r_tok]) & (o < 4 * a[r_tok] + 2 * b[r_tok])
            off = o[sel] - 4 * a[r_tok][sel]
        else:
            sel = o >= 4 * a[r_tok] + 2 * b[r_tok]
            off = np.zeros(int(sel.sum()), dtype=np.int64)
        blk_u = np.repeat(u, nb)
        tok_block = base[:-1][r_tok[sel]] + off // bs
        tok_within = off % bs
        out[bs] = (blk_u, np.flatnonzero(sel), tok_block, tok_within)
    return out


def _pad_shard(tbl, c):
    """[VS, D] f32 shard c of tbl, zero-padded to [VSP, D], as flat [128, VSP]."""
    out = np.zeros((VSP, D), dtype=np.float32)
    out[:VS] = tbl[c * VS : (c + 1) * VS]
    return out.reshape(128, VSP)


def kernel(**inputs):
    from concourse.bass_utils import run_bass_kernel_spmd

    x = np.asarray(inputs["x"])
    w_mu = np.ascontiguousarray(inputs["W_mu"], dtype=np.float32)
    w_rho = np.ascontiguousarray(inputs["W_rho"], dtype=np.float32)
    eps = np.ascontiguousarray(inputs["eps"], dtype=np.float32)

    xf = x.reshape(-1).astype(np.int64, copy=False)
    n_tok = xf.size
    # Stable sort by global row == bucket by owning core AND sort by row.
    order = np.argsort(xf, kind="stable")
    xs = xf[order]
    offs = np.searchsorted(xs, np.arange(NCORES + 1) * VS)

    per_core = [
        _encode_blocks(xs[offs[c] : offs[c + 1]] - c * VS) for c in range(NCORES)
    ]
    # Split blocks into lo/hi halves of the flat layout: row u sits at
    # partition u // RPP, q = u % RPP; half = q >= HQ.
    split = []  # per core: {bs: (hi_mask, n_lo, n_hi, u_dev)}
    for c in range(NCORES):
        sc = {}
        for bs in STREAMS:
            blk_u = per_core[c][bs][0]
            q = blk_u % RPP
            hi = q >= HQ
            u_dev = (blk_u // RPP) * HQ + np.where(hi, q - HQ, q)
            sc[bs] = (hi, int((~hi).sum()), int(hi.sum()), u_dev.astype(np.int16))
        split.append(sc)
    nbps = {
        bs: (
            max(
                TBS[bs],
                math.ceil(max(split[c][bs][1] for c in range(NCORES)) / TBS[bs])
                * TBS[bs],
            ),
            max(
                TBS[bs],
                math.ceil(max(split[c][bs][2] for c in range(NCORES)) / TBS[bs])
                * TBS[bs],
            ),
        )
        for bs in STREAMS
    }

    in_maps = []
    slots = []  # per core {bs: flat device slot per selected token}
    for c in range(NCORES):
        m = {
            "mu": _pad_shard(w_mu, c),
            "rho": _pad_shard(w_rho, c),
            "eps": _pad_shard(eps, c),
        }
        sl_c = {}
        for bs in STREAMS:
            blk_u, tok_sel, tok_block, tok_within = per_core[c][bs]
            hi, n_lo, n_hi, u_dev = split[c][bs]
            lo_p, hi_p = nbps[bs]
            # new block position: lo blocks keep rank; hi blocks after pad.
            new_pos = np.empty(blk_u.size, dtype=np.int64)
            new_pos[~hi] = np.arange(n_lo)
            new_pos[hi] = lo_p + np.arange(n_hi)
            li = np.zeros(lo_p + hi_p, dtype=np.int16)
            li[new_pos] = u_dev
            m[f"idx{bs}"] = np.ascontiguousarray(np.tile(li.reshape(-1, 16).T, (8, 1)))
            sl_c[bs] = new_pos[tok_block] * bs + tok_within
        in_maps.append(m)
        slots.append(sl_c)

    nc = _get_nc(nbps)
    res = run_bass_kernel_spmd(nc, in_maps, core_ids=list(range(NCORES)), trace=TRACE)
    if TRACE:
        LAST_PROFILE["res"] = res

    out = np.empty((n_tok, D), dtype=np.float32)
    for c in range(NCORES):
        pos = order[offs[c] : offs[c + 1]]
        for bs in STREAMS:
            tok_sel = per_core[c][bs][1]
            dev = res.results[c][f"out{bs}"].reshape(-1, D)
            out[pos[tok_sel]] = dev[slots[c][bs]]
    return out.reshape(*x.shape, D)



# revision 2
# speedup vs baseline: 1.5580x; 1.5580x over previous
"""Bayesian-embedding lookup (BBBEmbedding) Trainium2 kernel, 8 NeuronCores.

reference:
    sampled = W_mu + log1p(exp(W_rho)) * clip(eps, -10, 10)   # [V, D]
    out     = sampled[x]                                      # [B, L, D]

Strategy (model-parallel row sharding + telescoping step-matrix gather):
  - Row-shard the tables across 8 cores (VS = 12500 rows -> 98 blocks of
    128 rows, zero-padded to 12544).
  - Host sorts the B*L token ids; each core's tokens bucket into its 98
    row-blocks.  Per-block token counts are padded to the max over the 8
    cores (so the compiled program is SPMD-uniform) rounded up to 64.
  - Per block, on device:
      dsamp[r,:] = sampled[r,:] - sampled[r-1,:] is computed directly from
      mu/eps by two PSUM-accumulated matmuls against a static bidiagonal
      B^T (the softplus(rho)=const scale is folded into B^T when W_rho is
      uniform; otherwise softplus/clip run on ACT/DVE first).
      S[r,t] = (t >= start_r) is a 0/1 step matrix built in ONE DVE
      tensor_scalar(is_ge) against an uploaded f32 iota, with per-row
      (per-partition) start offsets.  Because tokens are sorted,
      (dsamp^T @ S)[d,t] telescopes to sampled[row_t, d]: the gather IS a
      matmul, no descriptor generation, no DRAM round-trip.
      The PSUM result is copied once per block to bf16 staging (ACT/DVE
      alternating) and stored to DRAM by a casting SWDGE DMA (bf16->f32).
  - Device output is [128 d, T] (d-major); host transposes + un-permutes.
  - DMA per core ~69 MB (13 table + 56 out) vs ~150 MB for the
    gather-from-DRAM approach; SWDGE per-index descriptor cost is gone.
"""

import math

import numpy as np

V = 100000
D = 128
NCORES = 8
VS = V // NCORES  # 12500 rows per core
NBLK = 98  # 128-row blocks per core (98*128 = 12544 padded rows)
VSP = NBLK * 128
G = 7  # blocks per table-load DMA group / out-store group
NG = NBLK // G
CHUNK = 512  # max fp32 matmul moving free dim

_nc_cache: dict = {}

TRACE = False
LAST_PROFILE: dict = {}


def _build_nc(pbs, fast, num_devices=NCORES):
    """Build + compile the per-core Bass program.

    pbs: tuple of 98 per-block padded token counts (multiples of 64).
    fast: W_rho uniform and |eps|<=10 -> softplus/clip folded out.
    """
    import concourse.bacc as bacc
    import concourse.bass as bass
    import concourse.tile as tile
    from concourse import mybir

    f32 = mybir.dt.float32
    bf16 = mybir.dt.bfloat16
    Alu = mybir.AluOpType

    pbmax = max(pbs)
    seg_banks = 3 if pbmax <= 1536 else 2
    assert pbmax <= seg_banks * 512 * 2, f"block too large: {pbmax}"
    seg_w = seg_banks * 512
    ops_bufs = 2 if seg_banks == 3 else 3
    ob = np.concatenate([[0], np.cumsum(pbs)]).astype(int)  # out col offsets
    t_dev = int(ob[-1])
    tw = 256 if fast else 384  # interleaved per-(block,row) table width

    nc = bacc.Bacc(
        "TRN2", target_bir_lowering=False, debug=False, num_devices=num_devices
    )
    tbl_d = nc.dram_tensor("tbl", [VSP, tw], f32, kind="ExternalInput").ap()
    iota_d = nc.dram_tensor("iota", [128, pbmax], f32, kind="ExternalInput").ap()
    starts_d = nc.dram_tensor("starts", [128, NBLK], f32, kind="ExternalInput").ap()
    bts_d = nc.dram_tensor("bts", [128, 256], f32, kind="ExternalInput").ap()
    out_d = nc.dram_tensor("out", [128, t_dev], f32, kind="ExternalOutput").ap()

    with tile.TileContext(nc) as tc:
        with (
            tc.tile_pool(name="consts", bufs=1) as const_pool,
            tc.tile_pool(name="tblp", bufs=3) as tbl_pool,
            tc.tile_pool(name="sp", bufs=3) as s_pool,
            tc.tile_pool(name="dssb", bufs=3) as ds_pool,
            tc.tile_pool(name="stagep", bufs=2) as stage_pool,
            tc.tile_pool(name="workp", bufs=3) as work_pool,
            tc.tile_pool(name="dsps", bufs=2, space="PSUM") as dsps_pool,
            tc.tile_pool(name="ops", bufs=ops_bufs, space="PSUM") as ops_pool,
        ):
            iota_t = const_pool.tile([128, pbmax], f32, tag="iota")
            starts_t = const_pool.tile([128, NBLK], f32, tag="starts")
            bts_t = const_pool.tile([128, 256], f32, tag="bts")
            nc.gpsimd.dma_start(out=iota_t[:], in_=iota_d[:])
            nc.gpsimd.dma_start(out=starts_t[:], in_=starts_d[:])
            nc.gpsimd.dma_start(out=bts_t[:], in_=bts_d[:])

            stage_max = max(int(ob[g * G + G] - ob[g * G]) for g in range(NG))
            ci = 0  # copy round-robin counter
            for g in range(NG):
                b0 = g * G
                tbl_t = tbl_pool.tile([128, G, tw], f32, tag="tbl")
                src = bass.AP(
                    tensor=tbl_d.tensor,
                    offset=b0 * 128 * tw,
                    ap=[[tw, 128], [128 * tw, G], [1, tw]],
                )
                nc.sync.dma_start(out=tbl_t[:], in_=src)
                sw = int(ob[b0 + G] - ob[b0])
                stage_t = stage_pool.tile([128, stage_max], bf16, tag="stage")
                for j in range(G):
                    b = b0 + j
                    pb = int(pbs[b])
                    col = int(ob[b] - ob[b0])
                    mu_ap = tbl_t[:, j, 0:128]
                    dps = dsps_pool.tile([128, 128], f32, tag="dps")
                    if fast:
                        eps_ap = tbl_t[:, j, 128:256]
                        nc.tensor.matmul(
                            dps[:], lhsT=bts_t[:, 0:128], rhs=eps_ap,
                            start=True, stop=False,
                        )
                    else:
                        rho_ap = tbl_t[:, j, 128:256]
                        eps_ap = tbl_t[:, j, 256:384]
                        sig_t = work_pool.tile([128, 128], f32, tag="sig")
                        nc.scalar.activation(
                            out=sig_t[:], in_=rho_ap,
                            func=mybir.ActivationFunctionType.Softplus,
                        )
                        ceps_t = work_pool.tile([128, 128], f32, tag="ceps")
                        nc.vector.tensor_scalar(
                            out=ceps_t[:], in0=eps_ap,
                            scalar1=10.0, scalar2=-10.0,
                            op0=Alu.min, op1=Alu.max,
                        )
                        nc.vector.tensor_tensor(
                            out=ceps_t[:], in0=ceps_t[:], in1=sig_t[:],
                            op=Alu.mult,
                        )
                        nc.tensor.matmul(
                            dps[:], lhsT=bts_t[:, 128:256], rhs=ceps_t[:],
                            start=True, stop=False,
                        )
                    nc.tensor.matmul(
                        dps[:], lhsT=bts_t[:, 128:256], rhs=mu_ap,
                        start=False, stop=True,
                    )
                    ds_t = ds_pool.tile([128, 128], f32, tag="ds")
                    s_t = s_pool.tile([128, pbmax], f32, tag="s")
                    nc.vector.tensor_scalar(
                        out=s_t[:, :pb], in0=iota_t[:, :pb],
                        scalar1=starts_t[:, b : b + 1], scalar2=None,
                        op0=Alu.is_ge,
                    )
                    # copy engine split: DVE 2 of 5, ACT 3 of 5 (S-gen is DVE)
                    dve_out = ci % 5 < 2
                    ci += 1
                    if dve_out:
                        nc.scalar.copy(out=ds_t[:], in_=dps[:])
                    else:
                        nc.vector.tensor_copy(ds_t[:], dps[:])
                    for so in range(0, pb, seg_w):
                        swid = min(seg_w, pb - so)
                        ops_t = ops_pool.tile([128, seg_w], f32, tag="opst")
                        for o in range(0, swid, CHUNK):
                            w = min(CHUNK, swid - o)
                            nc.tensor.matmul(
                                ops_t[:, o : o + w],
                                lhsT=ds_t[:],
                                rhs=s_t[:, so + o : so + o + w],
                                start=True, stop=True,
                            )
                        dst = stage_t[:, col + so : col + so + swid]
                        if dve_out:
                            nc.vector.tensor_copy(dst, ops_t[:, :swid])
                        else:
                            nc.scalar.copy(out=dst, in_=ops_t[:, :swid])
                nc.gpsimd.dma_start(
                    out=out_d[:, int(ob[b0]) : int(ob[b0]) + sw],
                    in_=stage_t[:, :sw],
                )

    nc.compile()
    return nc


def _get_nc(pbs, fast):
    key = (pbs, fast)
    nc = _nc_cache.get(key)
    if nc is None:
        nc = _build_nc(pbs, fast)
        _nc_cache[key] = nc
    return nc


def _pad_rows(tbl, c):
    out = np.zeros((VSP, D), dtype=np.float32)
    out[:VS] = tbl[c * VS : (c + 1) * VS]
    return out


def kernel(**inputs):
    from concourse.bass_utils import run_bass_kernel_spmd

    x = np.asarray(inputs["x"])
    w_mu = np.ascontiguousarray(inputs["W_mu"], dtype=np.float32)
    w_rho = np.ascontiguousarray(inputs["W_rho"], dtype=np.float32)
    eps = np.ascontiguousarray(inputs["eps"], dtype=np.float32)

    rho0 = w_rho.flat[0]
    fast = bool(np.all(w_rho == rho0)) and float(np.abs(eps).max()) <= 10.0

    xf = x.reshape(-1).astype(np.int64, copy=False)
    n_tok = xf.size
    order = np.argsort(xf, kind="stable")
    xs = xf[order]
    offs = np.searchsorted(xs, np.arange(NCORES + 1) * VS)

    # Per-core row-run boundaries: sg_c[g] = first local-token index whose
    # shard row >= g.  Block b spans rows [128b, 128b+128).
    us, sgs, bcs = [], [], []
    grid = np.arange(VSP + 1)
    for c in range(NCORES):
        u = xs[offs[c] : offs[c + 1]] - c * VS
        sg = np.searchsorted(u, grid)
        us.append(u)
        sgs.append(sg)
        bcs.append(np.diff(sg[::128]))  # [98] per-block token counts
    bc_all = np.stack(bcs)  # [8, 98]
    pbs = tuple(int(v) for v in np.maximum(64, ((bc_all.max(0) + 63) // 64) * 64))
    ob = np.concatenate([[0], np.cumsum(pbs)]).astype(np.int64)
    t_dev = int(ob[-1])
    pbmax = max(pbs)

    sigma = np.float32(np.log1p(np.exp(np.float32(rho0))))
    bt = (np.eye(128) - np.eye(128, k=1)).astype(np.float32)
    bts = np.concatenate([sigma * bt, bt], axis=1)  # [128, 256]
    iota = np.ascontiguousarray(
        np.broadcast_to(np.arange(pbmax, dtype=np.float32), (128, pbmax))
    )

    tw = 256 if fast else 384
    in_maps = []
    for c in range(NCORES):
        tblv = np.zeros((VSP, tw), dtype=np.float32)
        tblv[:, 0:128] = _pad_rows(w_mu, c)
        if fast:
            tblv[:, 128:256] = _pad_rows(eps, c)
        else:
            tblv[:, 128:256] = _pad_rows(w_rho, c)
            tblv[:, 256:384] = _pad_rows(eps, c)
        sg = sgs[c]
        start_rel = sg[:VSP] - np.repeat(sg[: VSP : 128][:NBLK], 128)
        starts = np.ascontiguousarray(
            start_rel.reshape(NBLK, 128).T.astype(np.float32)
        )
        in_maps.append(
            {"tbl": tblv, "iota": iota, "starts": starts, "bts": bts}
        )

    nc = _get_nc(pbs, fast)
    res = run_bass_kernel_spmd(nc, in_maps, core_ids=list(range(NCORES)), trace=TRACE)
    if TRACE:
        LAST_PROFILE["res"] = res

    out = np.empty((n_tok, D), dtype=np.float32)
    for c in range(NCORES):
        dev = res.results[c]["out"]  # [128, t_dev] f32, d-major
        u = us[c]
        b = u >> 7
        tok_base = sgs[c][: VSP : 128][:NBLK]
        cols = ob[b] + (np.arange(u.size) - tok_base[b])
        devT = np.ascontiguousarray(dev.T)
        out[order[offs[c] : offs[c + 1]]] = devT[cols]
    return out.reshape(*x.shape, D)


# revision 10
# speedup vs baseline: 2.3003x; 1.4765x over previous
"""Bayesian-embedding lookup (BBBEmbedding) Trainium2 kernel, 8 NeuronCores.

reference:
    sampled = W_mu + log1p(exp(W_rho)) * clip(eps, -10, 10)   # [V, D]
    out     = sampled[x]                                      # [B, L, D]

Strategy (model-parallel row sharding + telescoping step-matrix gather):
  - Row-shard the tables across 8 cores (VS = 12500 rows -> 98 blocks of
    128 rows, zero-padded to 12544).
  - Host sorts the B*L token ids; each core's tokens bucket into its 98
    row-blocks.  Per-block token counts are padded to the max over the 8
    cores (so the compiled program is SPMD-uniform) rounded up to 64.
  - Per block, on device:
      dsamp[r,:] = sampled[r,:] - sampled[r-1,:] is computed directly from
      mu/eps by two PSUM-accumulated matmuls against a static bidiagonal
      B^T (the softplus(rho)=const scale is folded into B^T when W_rho is
      uniform; otherwise softplus/clip run on ACT/DVE first).
      S[r,t] = (t >= start_r) is a 0/1 step matrix built in ONE DVE
      tensor_scalar(is_ge) against an uploaded f32 iota, with per-row
      (per-partition) start offsets.  Because tokens are sorted,
      (dsamp^T @ S)[d,t] telescopes to sampled[row_t, d]: the gather IS a
      matmul, no descriptor generation, no DRAM round-trip.
      The PSUM result is copied once per block to bf16 staging (ACT/DVE
      alternating) and stored to DRAM by a casting SWDGE DMA (bf16->f32).
  - Device output is [128 d, T] (d-major); host transposes + un-permutes.
  - DMA per core ~69 MB (13 table + 56 out) vs ~150 MB for the
    gather-from-DRAM approach; SWDGE per-index descriptor cost is gone.
"""

import math

import numpy as np

V = 100000
D = 128
NCORES = 8
VS = V // NCORES  # 12500 rows per core
NBLK = 98  # 128-row blocks per core (98*128 = 12544 padded rows)
VSP = NBLK * 128
G = 7  # blocks per table-load DMA group / out-store group
NG = NBLK // G
CHUNK = 512  # max fp32 matmul moving free dim

_nc_cache: dict = {}

TRACE = False
LAST_PROFILE: dict = {}


def _build_nc(pbs, fast, num_devices=NCORES):
    """Build + compile the per-core Bass program.

    pbs: tuple of 98 per-block padded token counts (multiples of 64).
    fast: W_rho uniform and |eps|<=10 -> softplus/clip folded out.
    """
    import concourse.bacc as bacc
    import concourse.bass as bass
    import concourse.tile as tile
    from concourse import mybir

    f32 = mybir.dt.float32
    f32r = mybir.dt.float32r
    bf16 = mybir.dt.bfloat16
    Alu = mybir.AluOpType

    def r32(ap):
        return ap.bitcast(f32r)

    pbmax = max(pbs)
    seg_banks = 3 if pbmax <= 1536 else 2
    assert pbmax <= seg_banks * 512 * 2, f"block too large: {pbmax}"
    seg_w = seg_banks * 512
    ops_bufs = 2 if seg_banks == 3 else 3
    ob = np.concatenate([[0], np.cumsum(pbs)]).astype(int)  # out col offsets
    t_dev = int(ob[-1])
    tw = 256 if fast else 384  # interleaved per-(block,row) table width

    nc = bacc.Bacc(
        "TRN2", target_bir_lowering=False, debug=False, num_devices=num_devices
    )
    tbl_d = nc.dram_tensor("tbl", [VSP, tw], bf16, kind="ExternalInput").ap()
    iota_d = nc.dram_tensor("iota", [128, pbmax], f32, kind="ExternalInput").ap()
    starts_d = nc.dram_tensor("starts", [128, NBLK], f32, kind="ExternalInput").ap()
    bts_d = nc.dram_tensor("bts", [128, 256], bf16, kind="ExternalInput").ap()
    out_d = nc.dram_tensor("out", [128, t_dev], f32, kind="ExternalOutput").ap()

    with tile.TileContext(nc) as tc:
        with (
            tc.tile_pool(name="consts", bufs=1) as const_pool,
            tc.tile_pool(name="tblp", bufs=3) as tbl_pool,
            tc.tile_pool(name="sp", bufs=3) as s_pool,
            tc.tile_pool(name="dssb", bufs=3) as ds_pool,
            tc.tile_pool(name="stagep", bufs=2) as stage_pool,
            tc.tile_pool(name="workp", bufs=3) as work_pool,
            tc.tile_pool(name="dsps", bufs=2, space="PSUM") as dsps_pool,
            tc.tile_pool(name="ops", bufs=ops_bufs, space="PSUM") as ops_pool,
        ):
            iota_t = const_pool.tile([128, pbmax], f32, tag="iota")
            starts_t = const_pool.tile([128, NBLK], f32, tag="starts")
            bts_t = const_pool.tile([128, 256], bf16, tag="bts")
            nc.gpsimd.dma_start(out=iota_t[:], in_=iota_d[:])
            nc.gpsimd.dma_start(out=starts_t[:], in_=starts_d[:])
            nc.gpsimd.dma_start(out=bts_t[:], in_=bts_d[:])

            stage_max = max(int(ob[g * G + G] - ob[g * G]) for g in range(NG))
            ci = 0  # copy round-robin counter
            for g in range(NG):
                b0 = g * G
                tbl_t = tbl_pool.tile([128, G, tw], bf16, tag="tbl")
                src = bass.AP(
                    tensor=tbl_d.tensor,
                    offset=b0 * 128 * tw,
                    ap=[[tw, 128], [128 * tw, G], [1, tw]],
                )
                nc.sync.dma_start(out=tbl_t[:], in_=src)
                sw = int(ob[b0 + G] - ob[b0])
                stage_t = stage_pool.tile([128, stage_max], bf16, tag="stage")
                for j in range(G):
                    b = b0 + j
                    pb = int(pbs[b])
                    col = int(ob[b] - ob[b0])
                    mu_ap = tbl_t[:, j, 0:128]
                    dps = dsps_pool.tile([128, 128], f32, tag="dps")
                    if fast:
                        eps_ap = tbl_t[:, j, 128:256]
                        nc.tensor.matmul(
                            dps[:], lhsT=bts_t[:, 0:128], rhs=eps_ap,
                            start=True, stop=False,
                        )
                    else:
                        rho_ap = tbl_t[:, j, 128:256]
                        eps_ap = tbl_t[:, j, 256:384]
                        sig_t = work_pool.tile([128, 128], bf16, tag="sig")
                        nc.scalar.activation(
                            out=sig_t[:], in_=rho_ap,
                            func=mybir.ActivationFunctionType.Softplus,
                        )
                        ceps_t = work_pool.tile([128, 128], bf16, tag="ceps")
                        nc.vector.tensor_scalar(
                            out=ceps_t[:], in0=eps_ap,
                            scalar1=10.0, scalar2=-10.0,
                            op0=Alu.min, op1=Alu.max,
                        )
                        nc.vector.tensor_tensor(
                            out=ceps_t[:], in0=ceps_t[:], in1=sig_t[:],
                            op=Alu.mult,
                        )
                        nc.tensor.matmul(
                            dps[:], lhsT=bts_t[:, 128:256], rhs=ceps_t[:],
                            start=True, stop=False,
                        )
                    nc.tensor.matmul(
                        dps[:], lhsT=bts_t[:, 128:256], rhs=mu_ap,
                        start=False, stop=True,
                    )
                    ds_t = ds_pool.tile([128, 128], f32r, tag="ds")
                    s_t = s_pool.tile([128, pbmax], f32r, tag="s")
                    nc.vector.tensor_scalar(
                        out=s_t[:, :pb], in0=iota_t[:, :pb],
                        scalar1=starts_t[:, b : b + 1], scalar2=None,
                        op0=Alu.is_ge,
                    )
                    # copy engine split: DVE 2 of 5, ACT 3 of 5 (S-gen is DVE)
                    dve_out = ci % 5 < 2
                    ci += 1
                    if dve_out:
                        nc.scalar.copy(out=ds_t[:], in_=dps[:])
                    else:
                        nc.vector.tensor_copy(ds_t[:], dps[:])
                    for so in range(0, pb, seg_w):
                        swid = min(seg_w, pb - so)
                        ops_t = ops_pool.tile([128, seg_w], f32, tag="opst")
                        for o in range(0, swid, CHUNK):
                            w = min(CHUNK, swid - o)
                            nc.tensor.matmul(
                                ops_t[:, o : o + w],
                                lhsT=ds_t[:],
                                rhs=s_t[:, so + o : so + o + w],
                                start=True, stop=True,
                            )
                        dst = stage_t[:, col + so : col + so + swid]
                        if dve_out:
                            nc.vector.tensor_copy(dst, ops_t[:, :swid])
                        else:
                            nc.scalar.copy(out=dst, in_=ops_t[:, :swid])
                nc.gpsimd.dma_start(
                    out=out_d[:, int(ob[b0]) : int(ob[b0]) + sw],
                    in_=stage_t[:, :sw],
                )

    nc.compile()
    return nc


def _get_nc(pbs, fast):
    key = (pbs, fast)
    nc = _nc_cache.get(key)
    if nc is None:
        nc = _build_nc(pbs, fast)
        _nc_cache[key] = nc
    return nc


def _pad_rows(tbl, c):
    out = np.zeros((VSP, D), dtype=np.float32)
    out[:VS] = tbl[c * VS : (c + 1) * VS]
    return out


def kernel(**inputs):
    from concourse.bass_utils import run_bass_kernel_spmd

    x = np.asarray(inputs["x"])
    w_mu = np.ascontiguousarray(inputs["W_mu"], dtype=np.float32)
    w_rho = np.ascontiguousarray(inputs["W_rho"], dtype=np.float32)
    eps = np.ascontiguousarray(inputs["eps"], dtype=np.float32)

    rho0 = w_rho.flat[0]
    fast = bool(np.all(w_rho == rho0)) and float(np.abs(eps).max()) <= 10.0

    xf = x.reshape(-1).astype(np.int64, copy=False)
    n_tok = xf.size
    order = np.argsort(xf, kind="stable")
    xs = xf[order]
    offs = np.searchsorted(xs, np.arange(NCORES + 1) * VS)

    # Per-core row-run boundaries: sg_c[g] = first local-token index whose
    # shard row >= g.  Block b spans rows [128b, 128b+128).
    us, sgs, bcs = [], [], []
    grid = np.arange(VSP + 1)
    for c in range(NCORES):
        u = xs[offs[c] : offs[c + 1]] - c * VS
        sg = np.searchsorted(u, grid)
        us.append(u)
        sgs.append(sg)
        bcs.append(np.diff(sg[::128]))  # [98] per-block token counts
    bc_all = np.stack(bcs)  # [8, 98]
    pbs = tuple(int(v) for v in np.maximum(64, ((bc_all.max(0) + 63) // 64) * 64))
    ob = np.concatenate([[0], np.cumsum(pbs)]).astype(np.int64)
    t_dev = int(ob[-1])
    pbmax = max(pbs)

    from concourse import mybir as _mybir

    bf16_np = _mybir.dt.np(_mybir.dt.bfloat16)
    sigma = np.float32(np.log1p(np.exp(np.float32(rho0))))
    bt = (np.eye(128) - np.eye(128, k=1)).astype(np.float32)
    bts = np.concatenate([sigma * bt, bt], axis=1).astype(bf16_np)  # [128, 256]
    iota = np.ascontiguousarray(
        np.broadcast_to(np.arange(pbmax, dtype=np.float32), (128, pbmax))
    )

    tw = 256 if fast else 384
    in_maps = []
    for c in range(NCORES):
        tblv = np.zeros((VSP, tw), dtype=bf16_np)
        tblv[:, 0:128] = _pad_rows(w_mu, c).astype(bf16_np)
        if fast:
            tblv[:, 128:256] = _pad_rows(eps, c).astype(bf16_np)
        else:
            tblv[:, 128:256] = _pad_rows(w_rho, c).astype(bf16_np)
            tblv[:, 256:384] = _pad_rows(eps, c).astype(bf16_np)
        sg = sgs[c]
        start_rel = sg[:VSP] - np.repeat(sg[: VSP : 128][:NBLK], 128)
        starts = np.ascontiguousarray(
            start_rel.reshape(NBLK, 128).T.astype(np.float32)
        )
        in_maps.append(
            {"tbl": tblv, "iota": iota, "starts": starts, "bts": bts}
        )

    nc = _get_nc(pbs, fast)
    res = run_bass_kernel_spmd(nc, in_maps, core_ids=list(range(NCORES)), trace=TRACE)
    if TRACE:
        LAST_PROFILE["res"] = res

    out = np.empty((n_tok, D), dtype=np.float32)
    for c in range(NCORES):
        dev = res.results[c]["out"]  # [128, t_dev] f32, d-major
        u = us[c]
        b = u >> 7
        tok_base = sgs[c][: VSP : 128][:NBLK]
        cols = ob[b] + (np.arange(u.size) - tok_base[b])
        devT = np.ascontiguousarray(dev.T)
        out[order[offs[c] : offs[c + 1]]] = devT[cols]
    return out.reshape(*x.shape, D)
